# revision 9
# baseline (speedup 1.0000x reference)
"""Multi-head self-attention Trainium2 kernel (B=8, S=1024, D=768, H=12, Hd=64).

Sharding: pure data-parallel, one batch element per NeuronCore (8 cores), no
collectives. Per core the full attention block runs SBUF-resident:

  x[1024,768] -> xT (PE transpose) -> qkT[12x(128,1024)] (transposed layout) and
  v' (natural layout, 65-col head blocks with a ones column for softmax denom)
  -> per head-pair: scoresT = kT.T@qT (K=64, two heads packed in the PE array
  via row tiling) -> exp on ACT (scale=1/8, no max subtraction: logits ~N(0,1))
  -> PV: outT'[65,1024] = v'.T @ expT (row 64 = softmax denominator)
  -> reciprocal + partition_broadcast + DVE multiply -> outT (f32r)
  -> proj: y = outT.T @ w_proj + b_proj -> DRAM.

Matmul operands are float32r (TF32-like, full PE speed at N>=256, ~1.5e-4 rel
err) except the exp weights / V which are bf16.
"""

import numpy as np

B, S, D = 8, 1024, 768
H, Hd = 12, 64
D3 = 3 * D
N_CORES = 8
P = 128

_CACHE = {}


def _build_nc():
    import concourse.bass as bass
    import concourse.mybir as mybir
    from concourse import bacc
    from concourse.tile import TileContext
    from concourse.masks import make_identity

    f32 = mybir.dt.float32
    f32r = mybir.dt.float32r
    bf16 = mybir.dt.bfloat16
    AF = mybir.ActivationFunctionType

    nc = bacc.Bacc("TRN2", target_bir_lowering=False, debug=False,
                   num_devices=N_CORES)

    x_d = nc.declare_dram_parameter("x", [S, D], f32, isOutput=False)
    wqkv_d = nc.declare_dram_parameter("w_qkv", [D, D3], f32, isOutput=False)
    bqkv_d = nc.declare_dram_parameter("b_qkv", [D3], f32, isOutput=False)
    wproj_d = nc.declare_dram_parameter("w_proj", [D, D], f32, isOutput=False)
    bproj_d = nc.declare_dram_parameter("b_proj", [D], f32, isOutput=False)
    out_d = nc.declare_dram_parameter("out", [S, D], f32, isOutput=True)

    KD = D // P            # 6 k-chunks of 128 over D
    ST = S // P            # 8 s-tiles of 128
    NPAIR = H // 2         # 6 head pairs

    with TileContext(nc) as tc:
        with tc.tile_pool(name="consts", bufs=1) as consts, \
             tc.tile_pool(name="big", bufs=1) as big:

            # ---------------- constants / biases ----------------
            ident = consts.tile([P, P], f32)
            make_identity(nc, ident[:])
            # b_qkv q/k part as columns: bqk_cols[p, j] = b_qkv[128j + p]
            bqk_cols = consts.tile([P, 12], f32)
            nc.sync.dma_start(out=bqk_cols[:],
                              in_=bqkv_d[0:12 * P].rearrange("(j p) -> p j", p=P))
            # v-part and proj biases broadcast across partitions
            brow = consts.tile([2, D], f32)
            nc.sync.dma_start(out=brow[0:1, :], in_=bqkv_d[2 * D:3 * D][None, :])
            nc.sync.dma_start(out=brow[1:2, :], in_=bproj_d[:][None, :])
            bv_bc = consts.tile([P, D], f32)
            nc.gpsimd.partition_broadcast(bv_bc[:], brow[0:1, :], channels=P)
            bp_row = consts.tile([1, D], f32)
            nc.sync.dma_start(out=bp_row[:], in_=bproj_d[:][None, :])
            bp_bc = consts.tile([P, D], f32)
            nc.gpsimd.partition_broadcast(bp_bc[:], bp_row[:], channels=P)

            # persistent attention-phase tensors (created lazily for SBUF reuse)
            qkT = [big.tile([P, S], f32r, name=f"qkT{mt}") for mt in range(12)]
            v_sb = [big.tile([P, 65 * H], bf16, name=f"v{st}") for st in range(ST)]

            # ============ phase-scoped: x, xT, w_qkv (freed after qkv) ========
            with tc.tile_pool(name="qkvpool", bufs=1) as qp, \
                 tc.tile_pool(name="xpool", bufs=3) as xpool, \
                 tc.tile_pool(name="psA", bufs=1, space="PSUM") as ps:
                # ---------------- weight loads (cast f32 -> f32r) -------------
                wq_sb = []   # 6 x [128, 2304] f32r  (w_qkv rows chunk)
                for kd in range(KD):
                    t = qp.tile([P, D3], f32r, name=f"wqkv{kd}")
                    nc.gpsimd.dma_start(out=t[:], in_=wqkv_d[kd * P:(kd + 1) * P, :])
                    wq_sb.append(t)

                # ---------------- x load + transpose -> xT (f32r) -------------
                xT = [qp.tile([P, S], f32r, name=f"xT{kd}") for kd in range(KD)]
                for si in range(ST):
                    xt = xpool.tile([P, D], f32, tag="x")
                    nc.sync.dma_start(out=xt[:], in_=x_d[si * P:(si + 1) * P, :])
                    for kd in range(KD):
                        pt = ps.tile([P, P], f32, tag="tp", bufs=4)
                        nc.tensor.transpose(pt[:], xt[:, kd * P:(kd + 1) * P], ident[:])
                        nc.vector.tensor_copy(xT[kd][:, si * P:(si + 1) * P], pt[:])

                # ---------------- qkT: [12][128, 1024] f32r -------------------
                for mt in range(12):
                    for st2 in range(2):
                        pq = ps.tile([P, 512], f32, tag="qkv", bufs=2)
                        for kd in range(KD):
                            nc.tensor.matmul(
                                pq[:], wq_sb[kd][:, mt * P:(mt + 1) * P],
                                xT[kd][:, st2 * 512:(st2 + 1) * 512],
                                start=(kd == 0), stop=(kd == KD - 1))
                        nc.vector.tensor_scalar_add(
                            qkT[mt][:, st2 * 512:(st2 + 1) * 512], pq[:],
                            bqk_cols[:, mt:mt + 1])

                # ------------- v' natural layout w/ ones cols (bf16) ----------
                # v_sb[st]: [128, 780]; head h at cols 65h..65h+63, col 65h+64 = 1
                for st in range(ST):
                    nc.gpsimd.memset(v_sb[st][:], 1.0)
                for st in range(ST):
                    for n0, nw, h0 in ((0, 512, 0), (512, 256, 8)):
                        pv = ps.tile([P, 512], f32, tag="qkv", bufs=2)
                        for kd in range(KD):
                            nc.tensor.matmul(
                                pv[:, 0:nw], xT[kd][:, st * P:(st + 1) * P],
                                wq_sb[kd][:, 2 * D + n0:2 * D + n0 + nw],
                                start=(kd == 0), stop=(kd == KD - 1))
                        nh = nw // Hd
                        nc.vector.tensor_add(
                            v_sb[st][:, 65 * h0:65 * h0 + 65 * nh]
                            .rearrange("p (h c) -> p h c", c=65)[:, :, 0:Hd],
                            pv[:, 0:nw].rearrange("p (h c) -> p h c", c=Hd),
                            bv_bc[:, n0:n0 + nw].rearrange("p (h c) -> p h c", c=Hd))

            # ---------------- attention + proj (phase-2 pools) ----------------
            _wp_cm = tc.tile_pool(name="wppool", bufs=1)
            wppool = _wp_cm.__enter__()
            _work_cm = tc.tile_pool(name="work", bufs=1)
            work = _work_cm.__enter__()
            _y_cm = tc.tile_pool(name="ypool", bufs=3)
            ypool = _y_cm.__enter__()
            _big2_cm = tc.tile_pool(name="big2", bufs=1)
            big2 = _big2_cm.__enter__()
            wp_sb = [wppool.tile([P, D], f32r, name=f"wproj{kd}") for kd in range(KD)]
            for kd in range(KD):
                nc.gpsimd.dma_start(out=wp_sb[kd][:], in_=wproj_d[kd * P:(kd + 1) * P, :])
            outT = [big2.tile([P, S], f32r, name=f"outT{p_i}") for p_i in range(NPAIR)]
            psB = tc.tile_pool(name="psB", bufs=1, space="PSUM")
            ps = psB.__enter__()
            for p_i in range(NPAIR):
                qt, kt = qkT[p_i], qkT[6 + p_i]
                # scoresT + exp, per sk tile
                expT = []
                for sk in range(ST):
                    et = work.tile([P, 2048], bf16, tag="expT", bufs=8,
                                   name=f"expT{p_i}_{sk}")
                    pscore = ps.tile([P, 2048], f32, tag="scores", bufs=1)
                    for hh in range(2):
                        lo, hi = hh * Hd, (hh + 1) * Hd
                        for sq in range(2):
                            nc.tensor.matmul(
                                pscore[:, hh * 1024 + sq * 512:hh * 1024 + (sq + 1) * 512],
                                kt[lo:hi, sk * P:(sk + 1) * P],
                                qt[lo:hi, sq * 512:(sq + 1) * 512],
                                start=True, stop=True)
                    nc.scalar.activation(et[:], pscore[:], AF.Exp, scale=float(Hd) ** -0.5)
                    expT.append(et)
                # PV: outT'_h [65, 1024] accumulated over sk
                po = []
                for hh in range(2):
                    h = 2 * p_i + hh
                    pvo = ps.tile([65, S], f32, tag="pv", bufs=2, name=f"pv{p_i}_{hh}")
                    for sk in range(ST):
                        for sq in range(2):
                            nc.tensor.matmul(
                                pvo[:, sq * 512:(sq + 1) * 512],
                                v_sb[sk][:, 65 * h:65 * h + 65],
                                expT[sk][:, hh * 1024 + sq * 512:hh * 1024 + (sq + 1) * 512],
                                start=(sk == 0), stop=(sk == ST - 1))
                    po.append(pvo)
                # softmax denominators -> reciprocal -> broadcast
                dpair = work.tile([2, S], f32, tag="dpair", bufs=1, name=f"dp{p_i}")
                d1 = work.tile([1, S], f32, tag="d1", bufs=1, name=f"d1_{p_i}")
                nc.vector.tensor_copy(dpair[0:1, :], po[0][64:65, :])
                nc.vector.tensor_copy(d1[:], po[1][64:65, :])
                nc.sync.dma_start(out=dpair[1:2, :], in_=d1[:, :])
                rpair = work.tile([2, S], f32, tag="rpair", bufs=1, name=f"rp{p_i}")
                nc.vector.reciprocal(rpair[:], dpair[:])
                r1 = work.tile([1, S], f32, tag="r1", bufs=1, name=f"r1_{p_i}")
                nc.sync.dma_start(out=r1[:, :], in_=rpair[1:2, :])
                bc0 = work.tile([Hd, S], f32, tag="bc0", bufs=1, name=f"bc0_{p_i}")
                bc1 = work.tile([Hd, S], f32, tag="bc1", bufs=1, name=f"bc1_{p_i}")
                nc.gpsimd.partition_broadcast(bc0[:], rpair[0:1, :], channels=Hd)
                nc.gpsimd.partition_broadcast(bc1[:], r1[0:1, :], channels=Hd)
                # normalize + pack pair tile (f32r)
                nc.vector.tensor_mul(outT[p_i][0:Hd, :], po[0][0:Hd, :], bc0[:])
                nc.vector.tensor_mul(outT[p_i][Hd:P, :], po[1][0:Hd, :], bc1[:])

            # ---------------- proj ----------------
            for st in range(ST):
                py = ps.tile([P, 2048], f32, tag="scores", bufs=1, name=f"py{st}")
                for n0, nw in ((0, 512), (512, 256)):
                    for k in range(NPAIR):
                        nc.tensor.matmul(
                            py[:, n0:n0 + nw],
                            outT[k][:, st * P:(st + 1) * P],
                            wp_sb[k][:, n0:n0 + nw],
                            start=(k == 0), stop=(k == NPAIR - 1))
                yt = ypool.tile([P, D], f32, tag="y")
                nc.vector.tensor_add(yt[:], py[:, 0:D], bp_bc[:])
                nc.sync.dma_start(out=out_d[st * P:(st + 1) * P, :], in_=yt[:])
            psB.__exit__(None, None, None)
            for _cm in (_big2_cm, _y_cm, _work_cm, _wp_cm):
                _cm.__exit__(None, None, None)

    nc.finalize()
    return nc


def _get_runner():
    """Build + compile once; return a callable(list_of_in_maps) -> list of out dicts."""
    if "runner" in _CACHE:
        return _CACHE["runner"]

    import jax
    import jax.numpy as jnp
    from jax.sharding import Mesh, PartitionSpec
    from jax.experimental.shard_map import shard_map
    import concourse.bass as bass
    import concourse.mybir as mybir
    from concourse import bass2jax
    from concourse.bass2jax import _bass_exec_p, install_neuronx_cc_hook, partition_id_tensor

    nc = _build_nc()
    install_neuronx_cc_hook()

    in_names = []
    out_names = []
    out_avals = []
    zero_out_shapes = []
    partition_name = nc.partition_id_tensor.name if nc.partition_id_tensor else None
    for alloc in nc.m.functions[0].allocations:
        if not isinstance(alloc, mybir.MemoryLocationSet):
            continue
        name = alloc.memorylocations[0].name
        if alloc.kind == "ExternalInput":
            if name != partition_name:
                in_names.append(name)
        elif alloc.kind == "ExternalOutput":
            out_names.append(name)
            shape = tuple(alloc.tensor_shape)
            dtype = mybir.dt.np(alloc.dtype)
            out_avals.append(jax.core.ShapedArray(shape, dtype))
            zero_out_shapes.append((shape, dtype))

    n_params = len(in_names)
    n_outs = len(out_avals)
    all_in_names = list(in_names) + list(out_names)
    if partition_name is not None:
        all_in_names.append(partition_name)
    donate = tuple(range(n_params, n_params + n_outs))

    def _body(*args):
        operands = list(args)
        if partition_name is not None:
            operands.append(partition_id_tensor())
        outs = _bass_exec_p.bind(
            *operands,
            out_avals=tuple(out_avals),
            in_names=tuple(all_in_names),
            out_names=tuple(out_names),
            lowering_input_output_aliases=(),
            sim_require_finite=True,
            sim_require_nnan=True,
            nc=nc,
        )
        return tuple(outs)

    devices = jax.devices()[:N_CORES]
    mesh = Mesh(np.asarray(devices), ("core",))
    in_specs = (PartitionSpec("core"),) * (n_params + n_outs)
    out_specs = (PartitionSpec("core"),) * n_outs
    sharded = jax.jit(
        shard_map(_body, mesh=mesh, in_specs=in_specs, out_specs=out_specs,
                  check_rep=False),
        donate_argnums=donate, keep_unused=True)

    def runner(in_maps):
        concat_in = [
            np.concatenate([np.asarray(in_maps[c][nm]) for c in range(N_CORES)], axis=0)
            for nm in in_names
        ]
        concat_zeros = [
            np.zeros((N_CORES * sh[0], *sh[1:]), dt) for sh, dt in zero_out_shapes
        ]
        out_arrs = sharded(*concat_in, *concat_zeros)
        out_arrs = [np.asarray(a) for a in out_arrs]
        return [
            {nm: out_arrs[i].reshape(N_CORES, *out_avals[i].shape)[c]
             for i, nm in enumerate(out_names)}
            for c in range(N_CORES)
        ]

    _CACHE["runner"] = runner
    return runner


def kernel(x, w_qkv, b_qkv, w_proj, b_proj):
    x = np.ascontiguousarray(np.asarray(x, dtype=np.float32))
    w_qkv = np.ascontiguousarray(np.asarray(w_qkv, dtype=np.float32))
    b_qkv = np.ascontiguousarray(np.asarray(b_qkv, dtype=np.float32))
    w_proj = np.ascontiguousarray(np.asarray(w_proj, dtype=np.float32))
    b_proj = np.ascontiguousarray(np.asarray(b_proj, dtype=np.float32))

    runner = _get_runner()
    in_maps = [
        {"x": x[c], "w_qkv": w_qkv, "b_qkv": b_qkv,
         "w_proj": w_proj, "b_proj": b_proj}
        for c in range(N_CORES)
    ]
    outs = runner(in_maps)
    return np.stack([outs[c]["out"] for c in range(N_CORES)], axis=0)


# revision 19
# speedup vs baseline: 2.2698x; 2.2698x over previous
"""Multi-head self-attention Trainium2 kernel (B=8, S=1024, D=768, H=12, Hd=64).

Sharding: pure data-parallel, one batch element per NeuronCore (8 cores), no
collectives. Per core the attention block runs SBUF-resident as one flat
pipeline (qkv projection, attention, and output projection overlap):

  x[1024,768] -> xT (PE transpose) -> qkT[12x(128,1024)] (transposed layout) and
  v' (natural layout, 65-col head blocks with a ones column for the softmax
  denominator) -> per head-pair: scoresT = kT.T@qT (K=64, two heads packed in
  the PE array via row tiling) -> exp on ACT (scale=1/8; no max subtraction:
  logits ~N(0,1)) -> PV: outT'[65,512] = v'.T @ expT (row 64 = denominator)
  -> reciprocal + partition_broadcast + DVE multiply -> outT (f32r)
  -> proj: y = outT.T @ w_proj + b_proj -> DRAM.

bf16 operands for qkv/scores/PV (inputs are ~N(0,1); measured end-to-end rel
err ~2-4e-3), float32r (TF32-like) for the final projection, fp32 PSUM
accumulation and fp32 softmax arithmetic throughout.
"""

import numpy as np

B, S, D = 8, 1024, 768
H, Hd = 12, 64
D3 = 3 * D
N_CORES = 8
P = 128

_CACHE = {}


def _build_nc():
    import concourse.bass as bass
    import concourse.mybir as mybir
    from concourse import bacc
    from concourse.tile import TileContext
    from concourse.masks import make_identity

    f32 = mybir.dt.float32
    f32r = mybir.dt.float32r
    bf16 = mybir.dt.bfloat16
    AF = mybir.ActivationFunctionType

    nc = bacc.Bacc("TRN2", target_bir_lowering=False, debug=False,
                   num_devices=N_CORES)

    x_d = nc.declare_dram_parameter("x", [S, D], f32, isOutput=False)
    wqkv_d = nc.declare_dram_parameter("w_qkv", [D, D3], f32, isOutput=False)
    bqkv_d = nc.declare_dram_parameter("b_qkv", [D3], f32, isOutput=False)
    wproj_d = nc.declare_dram_parameter("w_proj", [D, D], f32, isOutput=False)
    bproj_d = nc.declare_dram_parameter("b_proj", [D], f32, isOutput=False)
    out_d = nc.declare_dram_parameter("out", [S, D], f32, isOutput=True)

    KD = D // P            # 6 k-chunks of 128 over D
    ST = S // P            # 8 s-tiles of 128
    NPAIR = H // 2         # 6 head pairs

    with TileContext(nc) as tc:
        with tc.tile_pool(name="consts", bufs=1) as consts, \
             tc.tile_pool(name="big", bufs=1) as big, \
             tc.tile_pool(name="work", bufs=1) as work, \
             tc.tile_pool(name="ypool", bufs=3) as ypool, \
             tc.tile_pool(name="ps", bufs=1, space="PSUM") as ps:

            # ---------------- x load + transpose -> xT (bf16) ----------------
            # (emitted first: keeps gpsimd free so PE starts immediately)
            xT = [big.tile([P, S], bf16, name=f"xT{kd}") for kd in range(KD)]
            identf = consts.tile([P, P], f32)
            make_identity(nc, identf[:])
            for si in range(ST):
                xt = ypool.tile([P, D], f32, tag="x", bufs=3)
                nc.sync.dma_start(out=xt[:], in_=x_d[si * P:(si + 1) * P, :])
                for kd in range(KD):
                    pt = ps.tile([P, P], f32, tag="qkv", bufs=2)
                    nc.tensor.transpose(pt[:], xt[:, kd * P:(kd + 1) * P], identf[:])
                    nc.vector.tensor_copy(xT[kd][:, si * P:(si + 1) * P], pt[:])

            # ---------------- weight loads (cast f32 -> bf16 in DMA) ----------
            wq_sb = [big.tile([P, D3], bf16, name=f"wqkv{kd}") for kd in range(KD)]
            for kd in range(KD):
                nc.gpsimd.dma_start(out=wq_sb[kd][:], in_=wqkv_d[kd * P:(kd + 1) * P, :])

            # ---------------- biases ----------------
            bqk_cols = consts.tile([P, 12], f32)
            nc.sync.dma_start(out=bqk_cols[:],
                              in_=bqkv_d[0:12 * P].rearrange("(j p) -> p j", p=P))
            brow = ypool.tile([2, D], f32, tag="x", bufs=3, name="brow")
            nc.sync.dma_start(out=brow[0:1, :], in_=bqkv_d[2 * D:3 * D][None, :])
            bv_bc = consts.tile([P, D], f32)
            nc.gpsimd.partition_broadcast(bv_bc[:], brow[0:1, :], channels=P)
            bp_row = ypool.tile([1, D], f32, tag="x", bufs=3, name="bp_row")
            nc.sync.dma_start(out=bp_row[:], in_=bproj_d[:][None, :])
            bp_bc = consts.tile([P, D], f32)
            nc.gpsimd.partition_broadcast(bp_bc[:], bp_row[:], channels=P)

            qkT = [big.tile([P, S], bf16, name=f"qkT{mt}") for mt in range(12)]
            v_sb = [big.tile([P, 65 * H], bf16, name=f"v{st}") for st in range(ST)]
            outT = [big.tile([P, S], f32r, name=f"outT{p_i}") for p_i in range(NPAIR)]

            def emit_qkT_group(mt, st2):
                pq = ps.tile([P, 512], f32, tag="qkv", bufs=2,
                             name=f"pq{mt}_{st2}")
                for kd in range(KD):
                    nc.tensor.matmul(
                        pq[:], wq_sb[kd][:, mt * P:(mt + 1) * P],
                        xT[kd][:, st2 * 512:(st2 + 1) * 512],
                        start=(kd == 0), stop=(kd == KD - 1))
                nc.vector.tensor_scalar_add(
                    qkT[mt][:, st2 * 512:(st2 + 1) * 512], pq[:],
                    bqk_cols[:, mt:mt + 1])

            def emit_v():
                for st in range(ST):
                    nc.gpsimd.memset(v_sb[st][:], 1.0)
                for st in range(ST):
                    for n0, nw, h0 in ((0, 512, 0), (512, 256, 8)):
                        pv = ps.tile([P, 512], f32, tag="qkv", bufs=2,
                                     name=f"pvv{st}_{n0}")
                        for kd in range(KD):
                            nc.tensor.matmul(
                                pv[:, 0:nw], xT[kd][:, st * P:(st + 1) * P],
                                wq_sb[kd][:, 2 * D + n0:2 * D + n0 + nw],
                                start=(kd == 0), stop=(kd == KD - 1))
                        nh = nw // Hd
                        nc.vector.tensor_add(
                            v_sb[st][:, 65 * h0:65 * h0 + 65 * nh]
                            .rearrange("p (h c) -> p h c", c=65)[:, :, 0:Hd],
                            pv[:, 0:nw].rearrange("p (h c) -> p h c", c=Hd),
                            bv_bc[:, n0:n0 + nw].rearrange("p (h c) -> p h c", c=Hd))

            def pv_head(p_i, hh, expT):
                """PV for one head (sq-split accumulators), denominator copy,
                unnormalized copy to outT, then async recip+bcast+in-place mul."""
                h = 2 * p_i + hh
                r0 = hh * Hd
                dh = work.tile([1, S], f32, tag=f"dh{hh}", bufs=1,
                               name=f"dh{p_i}_{hh}")
                for sq in range(2):
                    pvo = ps.tile([65, 512], f32, tag="pv", bufs=2,
                                  name=f"pv{p_i}_{hh}_{sq}")
                    for sk in range(ST):
                        nc.tensor.matmul(
                            pvo[:],
                            v_sb[sk][:, 65 * h:65 * h + 65],
                            expT[sk][:, hh * 1024 + sq * 512:hh * 1024 + (sq + 1) * 512],
                            start=(sk == 0), stop=(sk == ST - 1))
                    nc.vector.tensor_copy(dh[0:1, sq * 512:(sq + 1) * 512],
                                          pvo[64:65, :])
                    nc.vector.tensor_copy(
                        outT[p_i][r0:r0 + Hd, sq * 512:(sq + 1) * 512],
                        pvo[0:Hd, :])
                nc.vector.reciprocal(dh[:], dh[:])
                bch = work.tile([P, S], f32, tag="bc", bufs=2,
                                name=f"bc{p_i}_{hh}")
                if hh == 0:
                    nc.gpsimd.partition_broadcast(bch[0:Hd, :], dh[0:1, :],
                                                  channels=Hd)
                else:
                    # gpsimd can only write from partition 0; bounce via DMA
                    btmp = work.tile([Hd, S], f32, tag="bctmp", bufs=2,
                                     name=f"bctmp{p_i}")
                    nc.gpsimd.partition_broadcast(btmp[:], dh[0:1, :],
                                                  channels=Hd)
                    nc.sync.dma_start(out=bch[Hd:P, :], in_=btmp[:, :])
                nc.vector.tensor_mul(outT[p_i][r0:r0 + Hd, :],
                                     outT[p_i][r0:r0 + Hd, :],
                                     bch[r0:r0 + Hd, :])

            def emit_pair(p_i, next_groups):
                """Scores+exp per sk with one next-wave qkT psum-group and
                PV(h0) interleaved per step; PV(h1) after."""
                qt, kt = qkT[p_i], qkT[6 + p_i]
                expT = []
                for sk in range(ST):
                    et = work.tile([P, 2048], bf16, tag="expT", bufs=8,
                                   name=f"expT{p_i}_{sk}")
                    for hh in range(2):
                        lo, hi = hh * Hd, (hh + 1) * Hd
                        pscore = ps.tile([P, 1024], f32, tag="scores", bufs=2,
                                         name=f"psc{p_i}_{sk}_{hh}")
                        for sq in range(2):
                            nc.tensor.matmul(
                                pscore[:, sq * 512:(sq + 1) * 512],
                                kt[lo:hi, sk * P:(sk + 1) * P],
                                qt[lo:hi, sq * 512:(sq + 1) * 512],
                                start=True, stop=True)
                        nc.scalar.activation(et[:, hh * 1024:(hh + 1) * 1024],
                                             pscore[:], AF.Exp,
                                             scale=float(Hd) ** -0.5)
                    expT.append(et)
                    if sk < len(next_groups):
                        emit_qkT_group(*next_groups[sk])
                pv_head(p_i, 0, expT)
                pv_head(p_i, 1, expT)

            # ---------------- interleaved emission ----------------
            wp_sb = [big.tile([P, D], f32r, name=f"wproj{kd}") for kd in range(KD)]
            emit_v()
            for mt, st2 in ((0, 0), (0, 1), (6, 0), (6, 1)):
                emit_qkT_group(mt, st2)
            for p_i in range(NPAIR):
                if p_i == 2:
                    for kd in range(KD):
                        nc.gpsimd.dma_start(out=wp_sb[kd][:],
                                            in_=wproj_d[kd * P:(kd + 1) * P, :])
                if p_i + 1 < NPAIR:
                    groups = [(p_i + 1, 0), (p_i + 1, 1), (7 + p_i, 0), (7 + p_i, 1)]
                else:
                    groups = []
                emit_pair(p_i, groups)

            # ---------------- proj ----------------
            for st in range(ST):
                yt = ypool.tile([P, D], f32, tag="y", bufs=2)
                for n0, nw in ((0, 512), (512, 256)):
                    ptag = "qkv" if n0 == 0 else "scores"
                    py = ps.tile([P, 512], f32, tag=ptag, bufs=2, name=f"py{st}_{n0}")
                    for k in range(NPAIR):
                        nc.tensor.matmul(
                            py[:, 0:nw],
                            outT[k][:, st * P:(st + 1) * P],
                            wp_sb[k][:, n0:n0 + nw],
                            start=(k == 0), stop=(k == NPAIR - 1))
                    nc.vector.tensor_add(yt[:, n0:n0 + nw], py[:, 0:nw],
                                         bp_bc[:, n0:n0 + nw])
                nc.sync.dma_start(out=out_d[st * P:(st + 1) * P, :], in_=yt[:])

    nc.finalize()
    return nc


def _get_runner():
    """Build + compile once; return a callable(list_of_in_maps) -> out dicts."""
    if "runner" in _CACHE:
        return _CACHE["runner"]

    import jax
    from jax.sharding import Mesh, PartitionSpec
    from jax.experimental.shard_map import shard_map
    import concourse.mybir as mybir
    from concourse.bass2jax import (_bass_exec_p, install_neuronx_cc_hook,
                                    partition_id_tensor)

    nc = _build_nc()
    install_neuronx_cc_hook()

    in_names = []
    out_names = []
    out_avals = []
    zero_out_shapes = []
    partition_name = nc.partition_id_tensor.name if nc.partition_id_tensor else None
    for alloc in nc.m.functions[0].allocations:
        if not isinstance(alloc, mybir.MemoryLocationSet):
            continue
        name = alloc.memorylocations[0].name
        if alloc.kind == "ExternalInput":
            if name != partition_name:
                in_names.append(name)
        elif alloc.kind == "ExternalOutput":
            out_names.append(name)
            shape = tuple(alloc.tensor_shape)
            dtype = mybir.dt.np(alloc.dtype)
            out_avals.append(jax.core.ShapedArray(shape, dtype))
            zero_out_shapes.append((shape, dtype))

    n_params = len(in_names)
    n_outs = len(out_avals)
    all_in_names = list(in_names) + list(out_names)
    if partition_name is not None:
        all_in_names.append(partition_name)
    donate = tuple(range(n_params, n_params + n_outs))

    def _body(*args):
        operands = list(args)
        if partition_name is not None:
            operands.append(partition_id_tensor())
        outs = _bass_exec_p.bind(
            *operands,
            out_avals=tuple(out_avals),
            in_names=tuple(all_in_names),
            out_names=tuple(out_names),
            lowering_input_output_aliases=(),
            sim_require_finite=True,
            sim_require_nnan=True,
            nc=nc,
        )
        return tuple(outs)

    devices = jax.devices()[:N_CORES]
    mesh = Mesh(np.asarray(devices), ("core",))
    in_specs = (PartitionSpec("core"),) * (n_params + n_outs)
    out_specs = (PartitionSpec("core"),) * n_outs
    sharded = jax.jit(
        shard_map(_body, mesh=mesh, in_specs=in_specs, out_specs=out_specs,
                  check_rep=False),
        donate_argnums=donate, keep_unused=True)

    def runner(in_maps):
        concat_in = [
            np.concatenate([np.asarray(in_maps[c][nm]) for c in range(N_CORES)],
                           axis=0)
            for nm in in_names
        ]
        concat_zeros = [
            np.zeros((N_CORES * sh[0], *sh[1:]), dt) for sh, dt in zero_out_shapes
        ]
        out_arrs = sharded(*concat_in, *concat_zeros)
        out_arrs = [np.asarray(a) for a in out_arrs]
        return [
            {nm: out_arrs[i].reshape(N_CORES, *out_avals[i].shape)[c]
             for i, nm in enumerate(out_names)}
            for c in range(N_CORES)
        ]

    _CACHE["runner"] = runner
    return runner


def kernel(x, w_qkv, b_qkv, w_proj, b_proj):
    x = np.ascontiguousarray(np.asarray(x, dtype=np.float32))
    w_qkv = np.ascontiguousarray(np.asarray(w_qkv, dtype=np.float32))
    b_qkv = np.ascontiguousarray(np.asarray(b_qkv, dtype=np.float32))
    w_proj = np.ascontiguousarray(np.asarray(w_proj, dtype=np.float32))
    b_proj = np.ascontiguousarray(np.asarray(b_proj, dtype=np.float32))

    runner = _get_runner()
    in_maps = [
        {"x": x[c], "w_qkv": w_qkv, "b_qkv": b_qkv,
         "w_proj": w_proj, "b_proj": b_proj}
        for c in range(N_CORES)
    ]
    outs = runner(in_maps)
    return np.stack([outs[c]["out"] for c in range(N_CORES)], axis=0)
